# revision 19
# baseline (speedup 1.0000x reference)
"""Multi-head attention TRN2 Bass kernel (8 NeuronCores, tensor-parallel).

Sharding: Megatron-style TP over (batch x head-group). 8 cores = 2 batches x 4
head-groups of 4 heads each. Each core computes its heads' Q/K/V projections,
masked-softmax attention, and a partial output projection; the host sums the 4
partials per batch (the TP unshard).

v3 scheduling notes:
  - Input DMAs are a few large multi-dim transfers (the sync queue's ~0.6us
    per-DMA issue rate starved the projections when every tile was its own
    DMA). Streams are ordered wq,qT,wk,kT,wv,vT,wo,keep0 so each projection's
    data lands just before the PE needs it.
  - Normalize uses the PE ones-broadcast again (GpSimd partition_broadcast
    costs a ~7us LOAD_LIB and blocks the in-order DVE queue), but its PE ops
    are emitted AFTER the next head's first scores so the tensor queue never
    waits on the DVE sums copy.
  - PSUM: scores 2x[128,1024] + ctx 2x[65,1024] (8 banks); the normalize
    broadcast and the out-proj tiles borrow slots from the scores ring.
  - au/am pools are deep (8) so ScalarE's event-semaphore waits are
    pre-satisfied; out-proj of m-half 0 runs inside the attention phase.
"""
import os
import sys

for p in ("/opt/trn_rl_repo",):
    if p not in sys.path:
        sys.path.insert(0, p)

from contextlib import ExitStack

import numpy as np

import concourse.bass as bass
import concourse.tile as tile
from concourse import bacc, mybir
from concourse.bass_utils import run_bass_kernel_spmd

F32 = mybir.dt.float32
F16 = mybir.dt.float16
EXP = mybir.ActivationFunctionType.Exp

B, M, N, E = 2, 2048, 2048, 1024  # batch, q-len, k-len, d_model
H, DK = 16, 64                    # heads, head dim
NCORES = 8
GROUPS = 4                        # head groups (cores per batch)
DLOC = (H // GROUPS) * DK         # 256 per-core projection width
HL = H // GROUPS                  # 4 local heads
ET = E // 128                     # 8 e-tiles
NT = N // 128                     # 16 n-tiles

# tunables (env-overridable for experiments)
DEPTH = int(os.environ.get("K_DEPTH", "2"))
S_BUFS = int(os.environ.get("K_SBUFS", "2"))
C_BUFS = int(os.environ.get("K_CBUFS", "2"))
AU_BUFS = int(os.environ.get("K_AUBUFS", "8"))
AM_BUFS = int(os.environ.get("K_AMBUFS", "8"))
NORM_AT = int(os.environ.get("K_NORMAT", "3"))  # defer norm into next head


def _etile(dram, lo, hi, width):
    """DRAM [E, width] rows lo*128..hi*128 viewed as [128, hi-lo, width]."""
    return dram[lo * 128 : hi * 128, :].rearrange("(a p) m -> p a m", p=128)


def build_program() -> bass.Bass:
    nc = bacc.Bacc()

    qT_d = nc.dram_tensor("qT", [E, M], F16, kind="ExternalInput")
    kT_d = nc.dram_tensor("kT", [E, N], F16, kind="ExternalInput")
    vT_d = nc.dram_tensor("vT", [E, N], F16, kind="ExternalInput")
    keepT_d = nc.dram_tensor("keepT", [N, M], F16, kind="ExternalInput")
    wqT_d = nc.dram_tensor("wqT", [E, DLOC], F16, kind="ExternalInput")
    wkT_d = nc.dram_tensor("wkT", [E, DLOC], F16, kind="ExternalInput")
    wvT_d = nc.dram_tensor("wvT", [E, DLOC], F16, kind="ExternalInput")
    woT_d = nc.dram_tensor("woT", [DLOC, E], F16, kind="ExternalInput")
    out_d = nc.dram_tensor("out", [M, E], F16, kind="ExternalOutput")

    with tile.TileContext(nc) as tc, ExitStack() as ctx:
        const_pool = ctx.enter_context(tc.tile_pool(name="const", bufs=1))
        w_pool = ctx.enter_context(tc.tile_pool(name="weights", bufs=1))
        act_pool = ctx.enter_context(tc.tile_pool(name="acts", bufs=1))

        ones64 = const_pool.tile([1, 64], F16)
        nc.vector.memset(ones64[:], 1.0)
        warm_exp = const_pool.tile([1, 64], F16)
        nc.scalar.activation(warm_exp[:], ones64[:], EXP, scale=0.125)

        wq_sb = w_pool.tile([128, ET, DLOC], F16, tag="wq")
        wk_sb = w_pool.tile([128, ET, DLOC], F16, tag="wk")
        wv_sb = w_pool.tile([128, ET, DLOC], F16, tag="wv")
        wo_sb = w_pool.tile([128, 2, E], F16, tag="wo")

        # qwz[hp][hl]: full-128-partition qw with the OTHER head's 64 rows
        # zeroed -> scores matmuls use K=128 (full PE rows) with a single
        # shared kw lhsT per (hp, nt).
        qwz = [
            [act_pool.tile([128, M], F16, tag=f"qwz{i}{j}", name=f"qwz{i}{j}")
             for j in range(2)]
            for i in range(2)
        ]
        for i in range(2):
            nc.vector.memset(qwz[i][0][bass.ts(1, 64), :], 0.0)
            nc.vector.memset(qwz[i][1][bass.ts(0, 64), :], 0.0)
        kw_sb = [act_pool.tile([128, N], F16, tag=f"kw{i}", name=f"kw{i}") for i in range(2)]
        # vw: [nt, head, 64 data + 1 ones]
        vw_sb = act_pool.tile([128, NT, HL, 65], F16, tag="vw")
        ctx_sb = [act_pool.tile([128, M], F16, tag=f"ctx{i}", name=f"ctx{i}") for i in range(2)]
        nc.vector.memset(vw_sb[:], 1.0)  # pre-fill ones cols; data cols overwritten

        keep0_pool = ctx.enter_context(tc.tile_pool(name="keep0", bufs=1))

        # ---- projections ----
        with (
            tc.tile_pool(name="xq", bufs=1) as xq_pool,
            tc.tile_pool(name="xk", bufs=1) as xk_pool,
            tc.tile_pool(name="xv", bufs=1) as xv_pool,
            tc.tile_pool(name="proj_ps", bufs=8, space="PSUM") as pps,
        ):
            qt = xq_pool.tile([128, ET, M], F16, tag="qt")
            kt = xk_pool.tile([128, ET, N], F16, tag="kt")
            vt = xv_pool.tile([128, ET, N], F16, tag="vt")

            # DMA order == the order the PE consumes the data; the first
            # chunks are small so compute starts while the DMA rings ramp
            nc.sync.dma_start(wq_sb[:, 0:2, :], _etile(wqT_d, 0, 2, DLOC))
            nc.sync.dma_start(qt[:, 0:1, :], _etile(qT_d, 0, 1, M))
            nc.sync.dma_start(wq_sb[:, 2:ET, :], _etile(wqT_d, 2, ET, DLOC))
            nc.sync.dma_start(qt[:, 1:2, :], _etile(qT_d, 1, 2, M))
            for c in range(1, 4):
                nc.sync.dma_start(
                    qt[:, 2 * c : 2 * c + 2, :], _etile(qT_d, 2 * c, 2 * c + 2, M)
                )
            nc.sync.dma_start(wk_sb[:], _etile(wkT_d, 0, ET, DLOC))
            for c in range(4):
                nc.sync.dma_start(
                    kt[:, 2 * c : 2 * c + 2, :], _etile(kT_d, 2 * c, 2 * c + 2, N)
                )
            nc.sync.dma_start(wv_sb[:], _etile(wvT_d, 0, ET, DLOC))
            for c in range(4):
                nc.sync.dma_start(
                    vt[:, 2 * c : 2 * c + 2, :], _etile(vT_d, 2 * c, 2 * c + 2, N)
                )
            nc.sync.dma_start(wo_sb[:], woT_d[:, :].rearrange("(a p) m -> p a m", p=128))

            keep_sbs = []
            ks0 = keep0_pool.tile([128, NT, 1024], F16, tag="keep", name="keep0")
            for qtr in range(4):
                nc.sync.dma_start(
                    ks0[:, qtr * 4 : qtr * 4 + 4, :],
                    keepT_d[qtr * 512 : qtr * 512 + 512, 0:1024].rearrange(
                        "(a p) m -> p a m", p=128
                    ),
                )
            keep_sbs.append(ks0)

            def proj_qk(xt, w_sb, writer):
                ps = [pps.tile([128, 512], F32, tag="pp", name=f"pp{j2}") for j2 in range(8)]
                for et in range(ET):
                    for d2 in range(2):
                        for mc in range(4):
                            nc.tensor.matmul(
                                ps[d2 * 4 + mc][:],
                                w_sb[:, et, d2 * 128 : (d2 + 1) * 128],
                                xt[:, et, bass.ts(mc, 512)],
                                start=(et == 0), stop=(et == ET - 1),
                            )
                for d2 in range(2):
                    for mc in range(4):
                        writer(d2, mc, ps[d2 * 4 + mc])

            def q_writer(d2, mc, ps):
                nc.vector.tensor_copy(
                    qwz[d2][0][bass.ts(0, 64), bass.ts(mc, 512)],
                    ps[bass.ts(0, 64), :],
                )
                nc.scalar.copy(
                    qwz[d2][1][bass.ts(1, 64), bass.ts(mc, 512)],
                    ps[bass.ts(1, 64), :],
                )

            def k_writer(d2, mc, ps):
                if (d2 * 4 + mc) % 2 == 0:
                    nc.vector.tensor_copy(kw_sb[d2][:, bass.ts(mc, 512)], ps[:])
                else:
                    nc.scalar.copy(kw_sb[d2][:, bass.ts(mc, 512)], ps[:])

            proj_qk(qt, wq_sb, q_writer)
            proj_qk(kt, wk_sb, k_writer)

            # v projection: et-outer in 2 groups of 8 n-tiles, so compute can
            # start as soon as the first vT chunk lands
            for g in range(2):
                ps = [pps.tile([128, 512], F32, tag="pp", name=f"vp{j2}") for j2 in range(8)]
                for et in range(ET):
                    for j in range(8):
                        nt = g * 8 + j
                        nc.tensor.matmul(
                            ps[j][:, 0:DLOC],
                            vt[:, et, bass.ts(nt, 128)],
                            wv_sb[:, et, :],
                            start=(et == 0), stop=(et == ET - 1),
                        )
                for j in range(8):
                    nt = g * 8 + j
                    dst = vw_sb[:, nt, :, 0:64]
                    src = ps[j][:, 0:DLOC].rearrange("p (h d) -> p h d", h=HL)
                    if j % 2 == 0:
                        nc.vector.tensor_copy(dst, src)
                    else:
                        nc.scalar.copy(dst, src)

        # ---- attention + overlapped output projection ----
        with (
            tc.tile_pool(name="keep1", bufs=1) as keep1_pool,
            tc.tile_pool(name="s_ps", bufs=S_BUFS, space="PSUM") as s_ps,
            tc.tile_pool(name="c_ps", bufs=C_BUFS, space="PSUM") as c_ps,
            tc.tile_pool(name="attn", bufs=AU_BUFS) as attn_pool,
            tc.tile_pool(name="attnm", bufs=AM_BUFS) as attnm_pool,
            tc.tile_pool(name="eps", bufs=2) as eps_pool,
            tc.tile_pool(name="o_sb", bufs=4) as o_sb_pool,
        ):
            ks1 = keep1_pool.tile([128, NT, 1024], F16, tag="keep", name="keep1")
            for hf in range(2):
                nc.sync.dma_start(
                    ks1[:, hf * 8 : hf * 8 + 8, :],
                    keepT_d[hf * 1024 : hf * 1024 + 1024, 1024:2048].rearrange(
                        "(a p) m -> p a m", p=128
                    ),
                )
            keep_sbs.append(ks1)

            def attn_head(mh, h, pending=None, last=False):
                """One head's scores/exp/mask/ctx. `pending` (the previous
                head's normalize tail) is emitted after NORM_AT score steps so
                the PE reaches it only when its DVE inputs are long done.
                Returns a closure that finishes THIS head's normalize; when
                `last`, the normalize is emitted immediately in 512-col chunks
                to minimize the out-projection's wait."""
                moff = mh * 1024
                keep_sb = keep_sbs[mh]
                hp, hl = divmod(h, 2)
                pctx = c_ps.tile([65, 1024], F32, tag="pctx")
                ams = {}
                for step in range(NT + DEPTH):
                    if step == NORM_AT and pending is not None:
                        pending()
                        pending = None
                    if step < NT:
                        nt = step
                        ps = s_ps.tile([128, 1024], F32, tag="ps")
                        for mc2 in range(2):
                            nc.tensor.matmul(
                                ps[:, bass.ts(mc2, 512)],
                                kw_sb[hp][:, bass.ts(nt, 128)],
                                qwz[hp][hl][
                                    :,
                                    moff + mc2 * 512 : moff + (mc2 + 1) * 512,
                                ],
                                start=True, stop=True,
                            )
                        au = attn_pool.tile([128, 1024], F16, tag="au")
                        nc.scalar.activation(au[:], ps[:], EXP, scale=0.125)
                        am = attnm_pool.tile([128, 1024], F16, tag="am")
                        nc.vector.tensor_mul(
                            am[:], au[:], keep_sb[:, nt, :]
                        )
                        ams[nt] = am
                    if step >= DEPTH:
                        nt = step - DEPTH
                        am = ams.pop(nt)
                        for mc2 in range(2):
                            nc.tensor.matmul(
                                pctx[:, bass.ts(mc2, 512)],
                                vw_sb[:, nt, h, :],
                                am[:, bass.ts(mc2, 512)],
                                start=(nt == 0), stop=(nt == NT - 1),
                            )
                if pending is not None:
                    pending()
                if last:
                    # latency-optimized: normalize in 512-col chunks so the
                    # out-projection's first tiles start after half the chain
                    for c2 in range(2):
                        sl = slice(c2 * 512, (c2 + 1) * 512)
                        sums_c = eps_pool.tile([1, 512], F16, tag="sums")
                        nc.vector.tensor_copy(sums_c[:], pctx[64:65, sl])
                        prb_c = s_ps.tile([64, 512], F32, tag="ps", name="prbc")
                        nc.tensor.matmul(
                            prb_c[:], ones64[:], sums_c[:], start=True, stop=True
                        )
                        rbs_c = eps_pool.tile([64, 512], F32, tag="rbs")
                        nc.vector.reciprocal_approx_fast(rbs_c[:], prb_c[:])
                        nc.vector.tensor_mul(
                            ctx_sb[hp][bass.ts(hl, 64), moff + c2 * 512 : moff + (c2 + 1) * 512],
                            pctx[0:64, sl],
                            rbs_c[:],
                        )
                    return None
                # sums leave PSUM right away (DVE); the rest is deferred so
                # the PE reaches the broadcast only when sums are long done
                sums = eps_pool.tile([1, 1024], F16, tag="sums")
                nc.vector.tensor_copy(sums[:], pctx[64:65, :])

                def finish():
                    prb = s_ps.tile([64, 1024], F32, tag="ps", name="prb")
                    for mc2 in range(2):
                        nc.tensor.matmul(
                            prb[:, bass.ts(mc2, 512)],
                            ones64[:],
                            sums[:, bass.ts(mc2, 512)],
                            start=True, stop=True,
                        )
                    rbs = eps_pool.tile([64, 1024], F32, tag="rbs")
                    nc.vector.reciprocal_approx_fast(rbs[:], prb[:])
                    nc.vector.tensor_mul(
                        ctx_sb[hp][bass.ts(hl, 64), moff : moff + 1024],
                        pctx[0:64, :],
                        rbs[:],
                    )

                return finish

            def out_proj(mh, pending=None):
                for i, mt in enumerate(range(mh * 8, mh * 8 + 8)):
                    if i == 1 and pending is not None:
                        pending()
                        pending = None
                    po = s_ps.tile([128, 1024], F32, tag="ps", name=f"po{mt}")
                    for ec in range(2):
                        for kt2 in range(2):
                            nc.tensor.matmul(
                                po[:, bass.ts(ec, 512)],
                                ctx_sb[kt2][:, bass.ts(mt, 128)],
                                wo_sb[:, kt2, ec * 512 : (ec + 1) * 512],
                                start=(kt2 == 0), stop=(kt2 == 1),
                            )
                    ob = o_sb_pool.tile([128, 1024], F16, tag="ob")
                    # ScalarE is idle during OUT blocks (no scores -> no exps)
                    if mt % 2 == 0:
                        nc.vector.tensor_copy(ob[:], po[:])
                    else:
                        nc.scalar.copy(ob[:], po[:])
                    nc.sync.dma_start(out_d[bass.ts(mt, 128), :], ob[:])
                return pending

            pending = None
            for h in range(HL):
                pending = attn_head(0, h, pending)
            pending = attn_head(1, 0, pending)
            pending = out_proj(0, pending)  # mh0 out while mh1 attends
            for h in range(1, HL - 1):
                pending = attn_head(1, h, pending)
            attn_head(1, HL - 1, pending, last=True)
            out_proj(1)

    nc.finalize()
    return nc


_PROGRAM = None


def _get_program():
    global _PROGRAM
    if _PROGRAM is None:
        _PROGRAM = build_program()
    return _PROGRAM


def _make_in_maps(q, k, v, mask, Wq, Wk, Wv, Wo):
    q = np.asarray(q, dtype=np.float32)
    k = np.asarray(k, dtype=np.float32)
    v = np.asarray(v, dtype=np.float32)
    mask = np.asarray(mask)
    Wq = np.asarray(Wq, dtype=np.float32)
    Wk = np.asarray(Wk, dtype=np.float32)
    Wv = np.asarray(Wv, dtype=np.float32)
    Wo = np.asarray(Wo, dtype=np.float32)

    per_batch = {}
    for b in range(B):
        per_batch[b] = dict(
            qT=np.ascontiguousarray(q[b].T.astype(np.float16)),
            kT=np.ascontiguousarray(k[b].T.astype(np.float16)),
            vT=np.ascontiguousarray(v[b].T.astype(np.float16)),
            keepT=np.ascontiguousarray(
                np.logical_not(mask[b]).T.astype(np.float16)
            ),
        )

    in_maps = []
    for c in range(NCORES):
        b, hg = divmod(c, GROUPS)
        sl = slice(hg * DLOC, (hg + 1) * DLOC)
        in_maps.append(
            dict(
                per_batch[b],
                wqT=np.ascontiguousarray(Wq[sl].T.astype(np.float16)),
                wkT=np.ascontiguousarray(Wk[sl].T.astype(np.float16)),
                wvT=np.ascontiguousarray(Wv[sl].T.astype(np.float16)),
                woT=np.ascontiguousarray(Wo[:, sl].T.astype(np.float16)),
            )
        )
    return in_maps


def _run(in_maps, trace=False):
    nc = _get_program()
    return run_bass_kernel_spmd(
        nc, in_maps, list(range(NCORES)), trace=trace
    )


def _assemble(results):
    out = np.zeros((B, M, E), dtype=np.float32)
    for c in range(NCORES):
        b = c // GROUPS
        out[b] += results[c]["out"].astype(np.float32)
    return out


def kernel(q, k, v, mask, Wq, Wk, Wv, Wo):
    in_maps = _make_in_maps(q, k, v, mask, Wq, Wk, Wv, Wo)
    res = _run(in_maps, trace=False)
    return _assemble(res.results)


def run_profiled(q, k, v, mask, Wq, Wk, Wv, Wo):
    """Like kernel(), but traces execution; returns (out, BassKernelResults)."""
    in_maps = _make_in_maps(q, k, v, mask, Wq, Wk, Wv, Wo)
    res = _run(in_maps, trace=True)
    return _assemble(res.results), res


# revision 26
# speedup vs baseline: 1.0050x; 1.0050x over previous
"""Multi-head attention TRN2 Bass kernel (8 NeuronCores, tensor-parallel).

Sharding: Megatron-style TP over (batch x head-group). 8 cores = 2 batches x 4
head-groups of 4 heads each. Each core computes its heads' Q/K/V projections,
masked-softmax attention, and a partial output projection; the host sums the 4
partials per batch (the TP unshard).

v3 scheduling notes:
  - Input DMAs are a few large multi-dim transfers (the sync queue's ~0.6us
    per-DMA issue rate starved the projections when every tile was its own
    DMA). Streams are ordered wq,qT,wk,kT,wv,vT,wo,keep0 so each projection's
    data lands just before the PE needs it.
  - Normalize uses the PE ones-broadcast again (GpSimd partition_broadcast
    costs a ~7us LOAD_LIB and blocks the in-order DVE queue), but its PE ops
    are emitted AFTER the next head's first scores so the tensor queue never
    waits on the DVE sums copy.
  - PSUM: scores 2x[128,1024] + ctx 2x[65,1024] (8 banks); the normalize
    broadcast and the out-proj tiles borrow slots from the scores ring.
  - au/am pools are deep (8) so ScalarE's event-semaphore waits are
    pre-satisfied; out-proj of m-half 0 runs inside the attention phase.
"""
import os
import sys

for p in ("/opt/trn_rl_repo",):
    if p not in sys.path:
        sys.path.insert(0, p)

from contextlib import ExitStack

import numpy as np

import concourse.bass as bass
import concourse.tile as tile
from concourse import bacc, mybir
from concourse.bass_utils import run_bass_kernel_spmd

F32 = mybir.dt.float32
F16 = mybir.dt.float16
I16 = mybir.dt.int16
EXP = mybir.ActivationFunctionType.Exp

B, M, N, E = 2, 2048, 2048, 1024  # batch, q-len, k-len, d_model
H, DK = 16, 64                    # heads, head dim
NCORES = 8
GROUPS = 4                        # head groups (cores per batch)
DLOC = (H // GROUPS) * DK         # 256 per-core projection width
HL = H // GROUPS                  # 4 local heads
ET = E // 128                     # 8 e-tiles
NT = N // 128                     # 16 n-tiles

# tunables (env-overridable for experiments)
DEPTH = int(os.environ.get("K_DEPTH", "2"))
S_BUFS = int(os.environ.get("K_SBUFS", "2"))
C_BUFS = int(os.environ.get("K_CBUFS", "2"))
AU_BUFS = int(os.environ.get("K_AUBUFS", "8"))
AM_BUFS = int(os.environ.get("K_AMBUFS", "8"))
NORM_AT = int(os.environ.get("K_NORMAT", "3"))  # defer norm into next head
# n-tiles whose exp runs on DVE via fp16 Schraudolph (x*a+b -> int16,
# bitcast fp16; ~1.8% relative error on those tiles) to relieve ScalarE,
# which otherwise paces the attention inner loop neck-and-neck with the PE
SCHRAU = int(os.environ.get("K_SCHRAU", "0"))  # 0: measured slower + worse err
SCHRAU_NTS = {5, 11} if SCHRAU >= 2 else ({11} if SCHRAU == 1 else set())
# exp(s/8) ~ bitcast_fp16(int16(s * (2^10/ln2)/8 + (15<<10) - 60))
SCH_MUL = 1477.3196 / 8.0
SCH_BIAS = 15360.0 - 60.0


def _etile(dram, lo, hi, width):
    """DRAM [E, width] rows lo*128..hi*128 viewed as [128, hi-lo, width]."""
    return dram[lo * 128 : hi * 128, :].rearrange("(a p) m -> p a m", p=128)


def build_program() -> bass.Bass:
    nc = bacc.Bacc()

    qT_d = nc.dram_tensor("qT", [E, M], F16, kind="ExternalInput")
    kT_d = nc.dram_tensor("kT", [E, N], F16, kind="ExternalInput")
    vT_d = nc.dram_tensor("vT", [E, N], F16, kind="ExternalInput")
    keepT_d = nc.dram_tensor("keepT", [N, M], F16, kind="ExternalInput")
    wqT_d = nc.dram_tensor("wqT", [E, DLOC], F16, kind="ExternalInput")
    wkT_d = nc.dram_tensor("wkT", [E, DLOC], F16, kind="ExternalInput")
    wvT_d = nc.dram_tensor("wvT", [E, DLOC], F16, kind="ExternalInput")
    woT_d = nc.dram_tensor("woT", [DLOC, E], F16, kind="ExternalInput")
    out_d = nc.dram_tensor("out", [M, E], F16, kind="ExternalOutput")

    with tile.TileContext(nc) as tc, ExitStack() as ctx:
        const_pool = ctx.enter_context(tc.tile_pool(name="const", bufs=1))
        w_pool = ctx.enter_context(tc.tile_pool(name="weights", bufs=1))
        act_pool = ctx.enter_context(tc.tile_pool(name="acts", bufs=1))

        ones64 = const_pool.tile([1, 64], F16)
        nc.vector.memset(ones64[:], 1.0)
        warm_exp = const_pool.tile([1, 64], F16)
        nc.scalar.activation(warm_exp[:], ones64[:], EXP, scale=0.125)

        wq_sb = w_pool.tile([128, ET, DLOC], F16, tag="wq")
        wk_sb = w_pool.tile([128, ET, DLOC], F16, tag="wk")
        wv_sb = w_pool.tile([128, ET, DLOC], F16, tag="wv")
        wo_sb = w_pool.tile([128, 2, E], F16, tag="wo")

        # qwz[hp][hl]: full-128-partition qw with the OTHER head's 64 rows
        # zeroed -> scores matmuls use K=128 (full PE rows) with a single
        # shared kw lhsT per (hp, nt).
        qwz = [
            [act_pool.tile([128, M], F16, tag=f"qwz{i}{j}", name=f"qwz{i}{j}")
             for j in range(2)]
            for i in range(2)
        ]
        for i in range(2):
            nc.vector.memset(qwz[i][0][bass.ts(1, 64), :], 0.0)
            nc.vector.memset(qwz[i][1][bass.ts(0, 64), :], 0.0)
        kw_sb = [act_pool.tile([128, N], F16, tag=f"kw{i}", name=f"kw{i}") for i in range(2)]
        # vw: [nt, head, 64 data + 1 ones]
        vw_sb = act_pool.tile([128, NT, HL, 65], F16, tag="vw")
        ctx_sb = [act_pool.tile([128, M], F16, tag=f"ctx{i}", name=f"ctx{i}") for i in range(2)]
        nc.vector.memset(vw_sb[:], 1.0)  # pre-fill ones cols; data cols overwritten

        keep0_pool = ctx.enter_context(tc.tile_pool(name="keep0", bufs=1))

        # ---- projections ----
        with (
            tc.tile_pool(name="xq", bufs=1) as xq_pool,
            tc.tile_pool(name="xk", bufs=1) as xk_pool,
            tc.tile_pool(name="xv", bufs=1) as xv_pool,
            tc.tile_pool(name="proj_ps", bufs=8, space="PSUM") as pps,
        ):
            qt = xq_pool.tile([128, ET, M], F16, tag="qt")
            kt = xk_pool.tile([128, ET, N], F16, tag="kt")
            vt = xv_pool.tile([128, ET, N], F16, tag="vt")

            # DMA order == the order the PE consumes the data; the first
            # chunks are small so compute starts while the DMA rings ramp
            nc.sync.dma_start(wq_sb[:, 0:2, :], _etile(wqT_d, 0, 2, DLOC))
            nc.sync.dma_start(qt[:, 0:1, :], _etile(qT_d, 0, 1, M))
            nc.sync.dma_start(wq_sb[:, 2:ET, :], _etile(wqT_d, 2, ET, DLOC))
            nc.sync.dma_start(qt[:, 1:2, :], _etile(qT_d, 1, 2, M))
            for c in range(1, 4):
                nc.sync.dma_start(
                    qt[:, 2 * c : 2 * c + 2, :], _etile(qT_d, 2 * c, 2 * c + 2, M)
                )
            nc.sync.dma_start(wk_sb[:], _etile(wkT_d, 0, ET, DLOC))
            for c in range(4):
                nc.sync.dma_start(
                    kt[:, 2 * c : 2 * c + 2, :], _etile(kT_d, 2 * c, 2 * c + 2, N)
                )
            nc.sync.dma_start(wv_sb[:], _etile(wvT_d, 0, ET, DLOC))
            for c in range(4):
                nc.sync.dma_start(
                    vt[:, 2 * c : 2 * c + 2, :], _etile(vT_d, 2 * c, 2 * c + 2, N)
                )
            nc.sync.dma_start(wo_sb[:], woT_d[:, :].rearrange("(a p) m -> p a m", p=128))

            keep_sbs = []
            ks0 = keep0_pool.tile([128, NT, 1024], F16, tag="keep", name="keep0")
            for qtr in range(4):
                nc.sync.dma_start(
                    ks0[:, qtr * 4 : qtr * 4 + 4, :],
                    keepT_d[qtr * 512 : qtr * 512 + 512, 0:1024].rearrange(
                        "(a p) m -> p a m", p=128
                    ),
                )
            keep_sbs.append(ks0)

            def proj_qk(xt, w_sb, writer):
                ps = [pps.tile([128, 512], F32, tag="pp", name=f"pp{j2}") for j2 in range(8)]
                for et in range(ET):
                    for d2 in range(2):
                        for mc in range(4):
                            nc.tensor.matmul(
                                ps[d2 * 4 + mc][:],
                                w_sb[:, et, d2 * 128 : (d2 + 1) * 128],
                                xt[:, et, bass.ts(mc, 512)],
                                start=(et == 0), stop=(et == ET - 1),
                            )
                for d2 in range(2):
                    for mc in range(4):
                        writer(d2, mc, ps[d2 * 4 + mc])

            def q_writer(d2, mc, ps):
                nc.vector.tensor_copy(
                    qwz[d2][0][bass.ts(0, 64), bass.ts(mc, 512)],
                    ps[bass.ts(0, 64), :],
                )
                nc.scalar.copy(
                    qwz[d2][1][bass.ts(1, 64), bass.ts(mc, 512)],
                    ps[bass.ts(1, 64), :],
                )

            def k_writer(d2, mc, ps):
                if (d2 * 4 + mc) % 2 == 0:
                    nc.vector.tensor_copy(kw_sb[d2][:, bass.ts(mc, 512)], ps[:])
                else:
                    nc.scalar.copy(kw_sb[d2][:, bass.ts(mc, 512)], ps[:])

            proj_qk(qt, wq_sb, q_writer)
            proj_qk(kt, wk_sb, k_writer)

            # v projection: et-outer in 2 groups of 8 n-tiles, so compute can
            # start as soon as the first vT chunk lands
            for g in range(2):
                ps = [pps.tile([128, 512], F32, tag="pp", name=f"vp{j2}") for j2 in range(8)]
                for et in range(ET):
                    for j in range(8):
                        nt = g * 8 + j
                        nc.tensor.matmul(
                            ps[j][:, 0:DLOC],
                            vt[:, et, bass.ts(nt, 128)],
                            wv_sb[:, et, :],
                            start=(et == 0), stop=(et == ET - 1),
                        )
                for j in range(8):
                    nt = g * 8 + j
                    dst = vw_sb[:, nt, :, 0:64]
                    src = ps[j][:, 0:DLOC].rearrange("p (h d) -> p h d", h=HL)
                    if j % 2 == 0:
                        nc.vector.tensor_copy(dst, src)
                    else:
                        nc.scalar.copy(dst, src)

        # ---- attention + overlapped output projection ----
        with (
            tc.tile_pool(name="keep1", bufs=1) as keep1_pool,
            tc.tile_pool(name="s_ps", bufs=S_BUFS, space="PSUM") as s_ps,
            tc.tile_pool(name="c_ps", bufs=C_BUFS, space="PSUM") as c_ps,
            tc.tile_pool(name="attn", bufs=AU_BUFS) as attn_pool,
            tc.tile_pool(name="attnm", bufs=AM_BUFS) as attnm_pool,
            tc.tile_pool(name="eps", bufs=2) as eps_pool,
            tc.tile_pool(name="o_sb", bufs=4) as o_sb_pool,
        ):
            ks1 = keep1_pool.tile([128, NT, 1024], F16, tag="keep", name="keep1")
            for hf in range(2):
                nc.sync.dma_start(
                    ks1[:, hf * 8 : hf * 8 + 8, :],
                    keepT_d[hf * 1024 : hf * 1024 + 1024, 1024:2048].rearrange(
                        "(a p) m -> p a m", p=128
                    ),
                )
            keep_sbs.append(ks1)

            def attn_head(mh, h, pending=None, last=False):
                """One head's scores/exp/mask/ctx. `pending` (the previous
                head's normalize tail) is emitted after NORM_AT score steps so
                the PE reaches it only when its DVE inputs are long done.
                Returns a closure that finishes THIS head's normalize; when
                `last`, the normalize is emitted immediately in 512-col chunks
                to minimize the out-projection's wait."""
                moff = mh * 1024
                keep_sb = keep_sbs[mh]
                hp, hl = divmod(h, 2)
                pctx = c_ps.tile([65, 1024], F32, tag="pctx")
                ams = {}
                for step in range(NT + DEPTH):
                    if step == NORM_AT and pending is not None:
                        pending()
                        pending = None
                    if step < NT:
                        nt = step
                        ps = s_ps.tile([128, 1024], F32, tag="ps")
                        for mc2 in range(2):
                            nc.tensor.matmul(
                                ps[:, bass.ts(mc2, 512)],
                                kw_sb[hp][:, bass.ts(nt, 128)],
                                qwz[hp][hl][
                                    :,
                                    moff + mc2 * 512 : moff + (mc2 + 1) * 512,
                                ],
                                start=True, stop=True,
                            )
                        if nt in SCHRAU_NTS:
                            au_i = attn_pool.tile([128, 1024], I16, tag="au", name="aui")
                            nc.vector.tensor_scalar(
                                au_i[:], ps[:], SCH_MUL, SCH_BIAS,
                                op0=mybir.AluOpType.mult, op1=mybir.AluOpType.add,
                            )
                            au_ap = au_i[:].bitcast(F16)
                        else:
                            au = attn_pool.tile([128, 1024], F16, tag="au")
                            nc.scalar.activation(au[:], ps[:], EXP, scale=0.125)
                            au_ap = au[:]
                        am = attnm_pool.tile([128, 1024], F16, tag="am")
                        nc.vector.tensor_mul(
                            am[:], au_ap, keep_sb[:, nt, :]
                        )
                        ams[nt] = am
                    if step >= DEPTH:
                        nt = step - DEPTH
                        am = ams.pop(nt)
                        for mc2 in range(2):
                            nc.tensor.matmul(
                                pctx[:, bass.ts(mc2, 512)],
                                vw_sb[:, nt, h, :],
                                am[:, bass.ts(mc2, 512)],
                                start=(nt == 0), stop=(nt == NT - 1),
                            )
                if pending is not None:
                    pending()
                if last:
                    # latency-optimized: normalize in 512-col chunks so the
                    # out-projection's first tiles start after half the chain
                    for c2 in range(2):
                        sl = slice(c2 * 512, (c2 + 1) * 512)
                        sums_c = eps_pool.tile([1, 512], F16, tag="sums")
                        nc.vector.tensor_copy(sums_c[:], pctx[64:65, sl])
                        prb_c = s_ps.tile([64, 512], F32, tag="ps", name="prbc")
                        nc.tensor.matmul(
                            prb_c[:], ones64[:], sums_c[:], start=True, stop=True
                        )
                        rbs_c = eps_pool.tile([64, 512], F32, tag="rbs")
                        nc.vector.reciprocal_approx_fast(rbs_c[:], prb_c[:])
                        nc.vector.tensor_mul(
                            ctx_sb[hp][bass.ts(hl, 64), moff + c2 * 512 : moff + (c2 + 1) * 512],
                            pctx[0:64, sl],
                            rbs_c[:],
                        )
                    return None
                # sums leave PSUM right away (DVE); the rest is deferred so
                # the PE reaches the broadcast only when sums are long done
                sums = eps_pool.tile([1, 1024], F16, tag="sums")
                nc.vector.tensor_copy(sums[:], pctx[64:65, :])

                def finish():
                    prb = s_ps.tile([64, 1024], F32, tag="ps", name="prb")
                    for mc2 in range(2):
                        nc.tensor.matmul(
                            prb[:, bass.ts(mc2, 512)],
                            ones64[:],
                            sums[:, bass.ts(mc2, 512)],
                            start=True, stop=True,
                        )
                    rbs = eps_pool.tile([64, 1024], F32, tag="rbs")
                    nc.vector.reciprocal_approx_fast(rbs[:], prb[:])
                    nc.vector.tensor_mul(
                        ctx_sb[hp][bass.ts(hl, 64), moff : moff + 1024],
                        pctx[0:64, :],
                        rbs[:],
                    )

                return finish

            def out_proj(mh, pending=None):
                for i, mt in enumerate(range(mh * 8, mh * 8 + 8)):
                    if i == 1 and pending is not None:
                        pending()
                        pending = None
                    # alternate PSUM rings: c_ps slots are free here (same
                    # 4KB/partition as ps), doubling effective ring depth so
                    # the first po doesn't wait on the previous head's last exp
                    pool = c_ps if i % 2 == 0 else s_ps
                    tag = "pctx" if i % 2 == 0 else "ps"
                    po = pool.tile([128, 1024], F32, tag=tag, name=f"po{mt}")
                    for ec in range(2):
                        for kt2 in range(2):
                            nc.tensor.matmul(
                                po[:, bass.ts(ec, 512)],
                                ctx_sb[kt2][:, bass.ts(mt, 128)],
                                wo_sb[:, kt2, ec * 512 : (ec + 1) * 512],
                                start=(kt2 == 0), stop=(kt2 == 1),
                            )
                    ob = o_sb_pool.tile([128, 1024], F16, tag="ob")
                    # ScalarE is idle during OUT blocks (no scores -> no exps)
                    if mt % 2 == 0:
                        nc.vector.tensor_copy(ob[:], po[:])
                    else:
                        nc.scalar.copy(ob[:], po[:])
                    nc.sync.dma_start(out_d[bass.ts(mt, 128), :], ob[:])
                return pending

            pending = None
            for h in range(HL):
                pending = attn_head(0, h, pending)
            pending = attn_head(1, 0, pending)
            pending = out_proj(0, pending)  # mh0 out while mh1 attends
            for h in range(1, HL - 1):
                pending = attn_head(1, h, pending)
            attn_head(1, HL - 1, pending, last=True)
            out_proj(1)

    nc.finalize()
    return nc


_PROGRAM = None


def _get_program():
    global _PROGRAM
    if _PROGRAM is None:
        _PROGRAM = build_program()
    return _PROGRAM


def _make_in_maps(q, k, v, mask, Wq, Wk, Wv, Wo):
    q = np.asarray(q, dtype=np.float32)
    k = np.asarray(k, dtype=np.float32)
    v = np.asarray(v, dtype=np.float32)
    mask = np.asarray(mask)
    Wq = np.asarray(Wq, dtype=np.float32)
    Wk = np.asarray(Wk, dtype=np.float32)
    Wv = np.asarray(Wv, dtype=np.float32)
    Wo = np.asarray(Wo, dtype=np.float32)

    per_batch = {}
    for b in range(B):
        per_batch[b] = dict(
            qT=np.ascontiguousarray(q[b].T.astype(np.float16)),
            kT=np.ascontiguousarray(k[b].T.astype(np.float16)),
            vT=np.ascontiguousarray(v[b].T.astype(np.float16)),
            keepT=np.ascontiguousarray(
                np.logical_not(mask[b]).T.astype(np.float16)
            ),
        )

    in_maps = []
    for c in range(NCORES):
        b, hg = divmod(c, GROUPS)
        sl = slice(hg * DLOC, (hg + 1) * DLOC)
        in_maps.append(
            dict(
                per_batch[b],
                wqT=np.ascontiguousarray(Wq[sl].T.astype(np.float16)),
                wkT=np.ascontiguousarray(Wk[sl].T.astype(np.float16)),
                wvT=np.ascontiguousarray(Wv[sl].T.astype(np.float16)),
                woT=np.ascontiguousarray(Wo[:, sl].T.astype(np.float16)),
            )
        )
    return in_maps


def _run(in_maps, trace=False):
    nc = _get_program()
    return run_bass_kernel_spmd(
        nc, in_maps, list(range(NCORES)), trace=trace
    )


def _assemble(results):
    out = np.zeros((B, M, E), dtype=np.float32)
    for c in range(NCORES):
        b = c // GROUPS
        out[b] += results[c]["out"].astype(np.float32)
    return out


def kernel(q, k, v, mask, Wq, Wk, Wv, Wo):
    in_maps = _make_in_maps(q, k, v, mask, Wq, Wk, Wv, Wo)
    res = _run(in_maps, trace=False)
    return _assemble(res.results)


def run_profiled(q, k, v, mask, Wq, Wk, Wv, Wo):
    """Like kernel(), but traces execution; returns (out, BassKernelResults)."""
    in_maps = _make_in_maps(q, k, v, mask, Wq, Wk, Wv, Wo)
    res = _run(in_maps, trace=True)
    return _assemble(res.results), res
